# revision 1
# baseline (speedup 1.0000x reference)
"""Block-circulant matmul kernel for 8 Trainium2 NeuronCores.

Reference op (per token row x of shape (4096,)):
    y = (x*d) @ M + bias,  M[(j,m),(i,n)] = W[i,j,(m-n)%256]  (circulant blocks)

Real-DFT factorization in three matmul stages per core, data-parallel over
batch (1024 tokens/core):
  stage1: per input block j, project onto the 256-col real DFT basis
  stage2: per frequency-group G (4 pair-slots), one 128x128 block-diag mix
  stage3: per output block i, inverse real DFT basis + bias
Between stages, two SBUF->SBUF partition-shuffle passes regroup the data
(frequency-major <-> block-major).  Each shuffle is decomposed per input
block j (resp. per group G) so it depends on only one stage-1 (stage-2)
step - the whole program is barrier-free dataflow, and each shuffle DMA
spreads its reads/writes across many SBUF ports (stride-16 partition
mappings).  All compute and IO in bf16 (PSUM accumulation fp32).

Self-contained: shapes hardcoded; no sibling imports.
"""
import os
import sys

for _p in ("/root/.axon_site", "/root/.axon_site/_ro/trn_rl_repo", "/root/.axon_site/_ro/pypackages"):
    if _p not in sys.path:
        sys.path.append(_p)

import numpy as np
import ml_dtypes

import concourse.bass as bass
import concourse.tile as tile
from concourse import bacc, mybir
from concourse import bass_utils

N_CORES = 8
B = 8192
D = 4096
BS = 256
K = 16               # blocks per side
NSLOT = BS // 2      # 128 frequency pair-slots
NT = B // N_CORES    # tokens per core (1024)
TH = 512             # psum half (max fp32 free dim per bank)

F32 = mybir.dt.float32
F32R = mybir.dt.float32r
BF16 = mybir.dt.bfloat16
BF16_NP = ml_dtypes.bfloat16

LAST_EXEC_NS = None
_CACHE = {}

# r = 2q+c indexes (slot-in-group q, component c); device partition layouts:
#   u  partition p = 16*r + gl   (gl = slot-group within pb-half)
#   u2 partition p = 16*r + j    (j  = input block)
#   v2 partition p = 16*r + i    (i  = output block)
#   v  partition p = 16*r + gl
# The shuffles are [128, NT] -> [8-strided (::16), 16*NT] folding DMAs
# (flat-order p -> (j+16*(p//16), p%16)); the strided narrow side spans 8
# SBUF ports instead of 2, and the wide side spans all 16.


# ---------------------------------------------------------------- host math

def _canonical_mats(W):
    m = np.arange(BS)
    T = np.zeros((BS, BS), np.float64)
    T[:, 0] = 1.0
    T[:, 1] = (-1.0) ** m
    for f in range(1, NSLOT):
        T[:, 2 * f] = np.cos(2 * np.pi * f * m / BS)
        T[:, 2 * f + 1] = np.sin(2 * np.pi * f * m / BS)

    Wf = np.fft.fft(W.astype(np.float64), axis=-1)
    p = Wf.real
    q = -Wf.imag

    jj = np.arange(K)
    M_slot = np.zeros((NSLOT, 2 * K, 2 * K), np.float64)
    for f in range(1, NSLOT):
        pf, qf = p[:, :, f], q[:, :, f]          # [i, j]
        M_slot[f][np.ix_(2 * jj, 2 * jj)] = pf.T
        M_slot[f][np.ix_(2 * jj + 1, 2 * jj)] = qf.T
        M_slot[f][np.ix_(2 * jj, 2 * jj + 1)] = qf.T
        M_slot[f][np.ix_(2 * jj + 1, 2 * jj + 1)] = -pf.T
    M_slot[0][np.ix_(2 * jj, 2 * jj)] = p[:, :, 0].T
    M_slot[0][np.ix_(2 * jj + 1, 2 * jj + 1)] = p[:, :, NSLOT].T

    n = np.arange(BS)
    R = np.zeros((BS, BS), np.float64)
    R[0, :] = 1.0 / BS
    R[1, :] = ((-1.0) ** n) / BS
    for f in range(1, NSLOT):
        R[2 * f, :] = 2.0 / BS * np.cos(2 * np.pi * f * n / BS)
        R[2 * f + 1, :] = -2.0 / BS * np.sin(2 * np.pi * f * n / BS)
    return T, M_slot, R


def _fft_host_mats(W, bias):
    T, M_slot, R = _canonical_mats(W)
    p_idx = np.arange(128)
    # partition p of u/v <-> gl = p%16, r = p//16, q = r//2, c = r%2
    gl_of = p_idx % 16
    q_of = (p_idx // 16) // 2
    c_of = (p_idx // 16) % 2

    # tb_dram (128, 4*128): [p_time, (mt*2+pb)*128+col] = T[mt*128+p_time, colmap(pb,col)]
    tb = np.zeros((128, 512), np.float64)
    for pb in range(2):
        slot = 64 * pb + 4 * gl_of + q_of
        cols = 2 * slot + c_of                    # canonical comp per device col
        for mt in range(2):
            tb[:, (mt * 2 + pb) * 128:(mt * 2 + pb + 1) * 128] = \
                T[mt * 128:(mt + 1) * 128, :][:, cols]

    # mix_dram (128, 32*128): [row, G*128+col]; rows/cols indexed 16*(2q+c)+k
    mix = np.zeros((128, 32 * 128), np.float64)
    kk = np.arange(K)
    for G in range(32):
        MG = np.zeros((128, 128), np.float64)
        for q in range(4):
            blk = M_slot[4 * G + q]
            for c in range(2):
                for cp in range(2):
                    MG[np.ix_(16 * (2 * q + c) + kk, 16 * (2 * q + cp) + kk)] = \
                        blk[np.ix_(2 * kk + c, 2 * kk + cp)]
        mix[:, G * 128:(G + 1) * 128] = MG

    # r_dram (128, 4*128): [p, (kt*2+nb)*128+col] = R[rowmap(kt,p), nb*128+col]
    # v[kt] partition p = 16*r + gl  (r = 2q+c)
    rd = np.zeros((128, 512), np.float64)
    for kt in range(2):
        rows = 2 * (64 * kt + 4 * gl_of + q_of) + c_of
        for nb in range(2):
            rd[:, (kt * 2 + nb) * 128:(kt * 2 + nb + 1) * 128] = \
                R[rows, :][:, nb * 128:(nb + 1) * 128]

    # bias (128, 32) f32: [p, i*2+nb] = bias[i*256 + nb*128 + p]
    bd = bias.astype(np.float64).reshape(K, 2, 128).transpose(2, 0, 1).reshape(128, 32)
    return (tb.astype(BF16_NP), mix.astype(BF16_NP), rd.astype(BF16_NP),
            np.ascontiguousarray(bd).astype(np.float32))


# ---------------------------------------------------------------- fft kernel

def _build_fft_nc():
    nc = bacc.Bacc("TRN2", target_bir_lowering=False, debug=False)
    # x_dev partition-major: [128, j*2048 + mt*1024 + t] (8KB runs per partition)
    xT = nc.dram_tensor("xT", [128, K * 2 * NT], BF16, kind="ExternalInput").ap()
    tb_d = nc.dram_tensor("tb", [128, 512], BF16, kind="ExternalInput").ap()
    mix_d = nc.dram_tensor("mix", [128, 32 * 128], BF16, kind="ExternalInput").ap()
    r_d = nc.dram_tensor("rmat", [128, 512], BF16, kind="ExternalInput").ap()
    bias_d = nc.dram_tensor("biasd", [128, 32], F32, kind="ExternalInput").ap()
    # y_dev partition-major: [128, ob*1024 + t] (8KB runs per 4-block store)
    yT = nc.dram_tensor("yT", [128, 32 * NT], BF16, kind="ExternalOutput").ap()

    ec = [0]

    def evac(dst, src, bias_ap=None):
        # alternate PSUM->SBUF evacuation between DVE and ACT
        if ec[0] % 2 == 0:
            if bias_ap is None:
                nc.vector.tensor_copy(dst, src)
            else:
                nc.vector.tensor_scalar_add(dst, src, bias_ap)
        else:
            if bias_ap is None:
                nc.scalar.copy(dst, src)
            else:
                nc.scalar.add(dst, src, bias_ap)
        ec[0] += 1

    with tile.TileContext(nc) as tc:
        with (
            tc.tile_pool(name="consts", bufs=1) as consts,
            tc.tile_pool(name="xpool", bufs=2) as xpool,
            tc.tile_pool(name="upool", bufs=2) as upool,      # serves u then v2
            tc.tile_pool(name="u2pool", bufs=2) as u2pool,
            tc.tile_pool(name="vpool", bufs=1) as vpool,
            tc.tile_pool(name="ypool", bufs=2) as ypool,
            tc.tile_pool(name="psA", bufs=2, space="PSUM") as psA,
            tc.tile_pool(name="psB", bufs=3, space="PSUM") as psB,
            tc.tile_pool(name="psC", bufs=3, space="PSUM") as psC,
        ):
            tb_sb = consts.tile([128, 512], BF16)
            nc.sync.dma_start(tb_sb[:], tb_d[:])
            mix_sb = consts.tile([128, 32 * 128], BF16)
            nc.sync.dma_start(mix_sb[:], mix_d[:])
            r_sb = consts.tile([128, 512], BF16)
            nc.sync.dma_start(r_sb[:], r_d[:])
            bias_sb = consts.tile([128, 32], F32)
            nc.sync.dma_start(bias_sb[:], bias_d[:])

            # ---- x loads (sync queue, HWDGE): 2 blocks per DMA ----
            x_t = {}
            for g in range(K // 2):
                xt = xpool.tile([128, 2 * 2 * NT], BF16, tag="x", name=f"x_{g}")
                nc.sync.dma_start(xt[:], xT[:, g * 4 * NT:(g + 1) * 4 * NT])
                x_t[g] = xt

            # ---- stage 1 + shuffle1, pipelined per block j ----
            # u[pb]: partition 16r+gl, free (j, t); u2[pb]: partition 16r+j, free (gl, t)
            u_sb = {}
            u2_sb = {}
            for pb in range(2):
                u_sb[pb] = upool.tile([128, K * NT], BF16, tag="u", name=f"u_{pb}")
                u2_sb[pb] = u2pool.tile([128, K * NT], BF16, tag="u2", name=f"u2_{pb}")
            for j in range(K):
                for pb in range(2):
                    for th in range(2):
                        ps1 = psA.tile([128, TH], F32, tag="ps1",
                                       name=f"ps1_{j}_{pb}_{th}")
                        for mt in range(2):
                            nc.tensor.matmul(
                                ps1[:],
                                tb_sb[:, (mt * 2 + pb) * 128:(mt * 2 + pb + 1) * 128],
                                x_t[j // 2][:, (j % 2) * 2 * NT + mt * NT + th * TH:
                                            (j % 2) * 2 * NT + mt * NT + (th + 1) * TH],
                                start=(mt == 0), stop=(mt == 1),
                            )
                        evac(u_sb[pb][:, j * NT + th * TH:j * NT + (th + 1) * TH],
                             ps1[:])
                    # shuffle1(pb, j): u2[16r + j, gl*NT+t] = u[16r+gl, j*NT+t]
                    # (strided dst = 8 ports; plain full-128 src = all ports)
                    _eng1 = (nc.sync, nc.scalar, nc.gpsimd)[(2 * j + pb) % 3]
                    _eng1.dma_start(
                        u2_sb[pb][j::16, :],
                        u_sb[pb][:, j * NT:(j + 1) * NT],
                    )

            # ---- stage 2 + shuffle2, pipelined per group G ----
            # v2[kt]: partition 16r+i, free (gl, t); v[kt]: partition 16r+gl, free (i,t)
            # v[1] reuses u2pool: u2[0] is dead once stage2's kt=0 half completes.
            v2_sb = {}
            for kt in range(2):
                v2_sb[kt] = upool.tile([128, K * NT], BF16, tag="u", name=f"v2_{kt}")
            v_sb = {0: vpool.tile([128, K * NT], BF16, tag="v", name="v_0")}
            for G in range(32):
                kt, gl = G // 16, G % 16
                if G == 16:
                    v_sb[1] = u2pool.tile([128, K * NT], BF16, tag="u2", name="v_1")
                for th in range(2):
                    ps2 = psB.tile([128, TH], F32, tag="ps2", name=f"ps2_{G}_{th}")
                    nc.tensor.matmul(
                        ps2[:],
                        mix_sb[:, G * 128:(G + 1) * 128],
                        u2_sb[kt][:, gl * NT + th * TH:gl * NT + (th + 1) * TH],
                        start=True, stop=True,
                    )
                    evac(v2_sb[kt][:, gl * NT + th * TH:gl * NT + (th + 1) * TH],
                         ps2[:])
                # shuffle2(kt, gl): v[kt][16r + gl, i*NT+t] = v2[kt][16r+i, gl*NT+t]
                _eng2 = (nc.sync, nc.scalar, nc.gpsimd)[G % 3]
                _eng2.dma_start(
                    v_sb[kt][gl::16, :],
                    v2_sb[kt][:, gl * NT:(gl + 1) * NT],
                )

            # ---- stage 3: per output block i, inverse basis + bias ----
            # y tiles hold 4 consecutive obs -> 8KB-per-partition stores
            y_t = None
            for i in range(K):
                for nb in range(2):
                    ob = i * 2 + nb
                    if ob % 4 == 0:
                        y_t = ypool.tile([128, 4 * NT], BF16, tag="y",
                                         name=f"y_{ob // 4}")
                    for th in range(2):
                        ps3 = psC.tile([128, TH], F32, tag="ps3", name=f"ps3_{ob}_{th}")
                        for kt in range(2):
                            nc.tensor.matmul(
                                ps3[:],
                                r_sb[:, (kt * 2 + nb) * 128:(kt * 2 + nb + 1) * 128],
                                v_sb[kt][:, i * NT + th * TH:
                                         i * NT + (th + 1) * TH],
                                start=(kt == 0), stop=(kt == 1),
                            )
                        evac(y_t[:, (ob % 4) * NT + th * TH:
                                 (ob % 4) * NT + (th + 1) * TH], ps3[:],
                             bias_sb[:, ob:ob + 1])
                    if ob % 4 == 3:
                        ig = ob // 4
                        nc.gpsimd.dma_start(
                            yT[:, ig * 4 * NT:(ig + 1) * 4 * NT], y_t[:])
    nc.compile()
    return nc


# ---------------------------------------------------------------- entry point

def _run(nc, in_maps):
    global LAST_EXEC_NS
    trace = bool(os.environ.get("BASS_TRACE"))
    res = bass_utils.run_bass_kernel_spmd(
        nc, in_maps, list(range(N_CORES)), trace=trace,
        tmpdir=os.environ.get("BASS_TRACE_DIR") or None,
    )
    LAST_EXEC_NS = res.exec_time_ns
    return res


def kernel(x, W, d_bernoulli, bias):
    x = np.asarray(x, dtype=np.float32)
    W = np.asarray(W, dtype=np.float32)
    d_bernoulli = np.asarray(d_bernoulli, dtype=np.float32)
    bias = np.asarray(bias, dtype=np.float32)

    xT = np.ascontiguousarray((x * d_bernoulli[None, :]).T)

    if "fft" not in _CACHE:
        _CACHE["fft"] = _build_fft_nc()
    tb, mix, rd, bd = _fft_host_mats(W, bias)
    in_maps = []
    for c in range(N_CORES):
        xs = xT[:, c * NT:(c + 1) * NT]                    # (D, NT)
        # device layout partition-major: [p, j*2048 + mt*1024 + t]
        xd = (xs.reshape(K, 2, 128, NT)
              .transpose(2, 0, 1, 3)
              .reshape(128, K * 2 * NT))
        in_maps.append({
            "xT": np.ascontiguousarray(xd).astype(BF16_NP),
            "tb": tb, "mix": mix, "rmat": rd, "biasd": bd,
        })
    res = _run(_CACHE["fft"], in_maps)

    out = np.empty((B, D), dtype=np.float32)
    for c in range(N_CORES):
        yv = np.asarray(res.results[c]["yT"]).reshape(128, 32, NT)
        out[c * NT:(c + 1) * NT, :] = (
            yv.transpose(2, 1, 0).reshape(NT, D).astype(np.float32))
    return out

